# revision 22
# baseline (speedup 1.0000x reference)
"""MoE (top-2 of 8 experts) Trainium2 kernel.

Strategy: expert-parallel across 8 NeuronCores with fractional-F tail
slot load balancing. Host computes the (tiny) router + top-2 token
dispatch. Each core runs the heavy expert FFN (x @ w1 -> gelu -> @ w2)
in bf16 with fp32 accumulation over:
  - an A slot: Ca tokens of its "home" expert, and
  - a tail slot: Cd overflow tokens of a heavy expert, computed over
    only nFf of the nF=24 F-tiles. A piece of overflow tokens is
    replicated across nF/nFf cores, each covering a disjoint F range;
    the host sums the partial outputs.
(Ca, nFf, Cd) are chosen at runtime from the actual expert loads to
minimize the uniform SPMD program cost. The fractional-F split matters
because the tensor queue has a ~66ns per-matmul issue floor: a full-F
extra chunk costs 288 instructions (19us) no matter how narrow, which
would exceed the padding it avoids; an nFf=6 tail chunk costs only 72.

Host applies the renormalized top-2 gates + b2 and scatter-adds the
per-slot outputs back into the full [B,S,D] output.

Shapes (hardcoded from the problem spec): B=4, S=2048, D=768, E=8,
F=4*D=3072, TOP_K=2.
"""

import os
import sys
import types

import numpy as np
import ml_dtypes

# concourse.bass_utils imports antenv.axon_hooks when tracing is requested
# (e.g. BASS_TRACE=1); some deployments lack that module. Provide a stub so
# tracing degrades gracefully (run without trace) instead of crashing.
try:
    from antenv import axon_hooks as _axon_hooks  # noqa: F401
except ImportError:
    _m = types.ModuleType("antenv.axon_hooks")
    _m._hook = None
    _m.set_axon_ntff_profile_hook = lambda h: setattr(_m, "_hook", h)
    _m.get_axon_ntff_profile_hook = lambda: _m._hook
    sys.modules["antenv.axon_hooks"] = _m
    try:
        import antenv

        antenv.axon_hooks = _m
    except ImportError:
        pass

import concourse.bass as bass
import concourse.tile as tile
from concourse import bacc, mybir
from concourse.bass_utils import run_bass_kernel_spmd

P = 128
D = 768
F = 3072
E = 8
TOP_K = 2
N_CORES = 8
nD, nF = D // P, F // P

bf16 = mybir.dt.bfloat16
f32 = mybir.dt.float32

# Stash of the most recent BassKernelResults (for test harness introspection).
last_results = None


def _chunks_of(total, size):
    """Split into chunks of `size`, avoiding a tail chunk under 256 when
    possible (small-N matmuls pay proportionally more issue overhead)."""
    out = []
    t0 = 0
    while t0 < total:
        rem = total - t0
        if size < rem < size + 256 and rem - 256 >= 256:
            out.append((t0, rem - 256))
            out.append((t0 + rem - 256, 256))
            break
        cs = min(size, rem)
        out.append((t0, cs))
        t0 += cs
    return out


def _build(Ca, nFf, Cd):
    """Expert FFN kernel: yT[D, Ca+Cd] = bf16((gelu(x @ w1 + b1) @ w2).T)

    Tokens [0, Ca) use the A weight set over all nF F-tiles; tokens
    [Ca, Ca+Cd) use the tail weight set over nFf F-tiles (a PARTIAL
    output the host sums across the piece's core group).
    Inputs arrive pre-transposed / pre-permuted so every contraction dim
    lands on SBUF partitions without any on-device transpose:
      xT[D, W]           x transposed (A tokens then tail tokens)
      w1pA[nF, P, nD*P]  w1pA[fi, p, d*P + c] = w1[d*P + p, fi*P + c]
      w2A[F, D]          natural layout
      b1pA[P, nF]        b1pA[p, o] = b1[o*P + p]
      w1pT/w2T/b1pT      same layout, nFf F-tiles only (when Cd > 0)
    Output is yT[dout, tok] in bf16 (host transposes back / upcasts).
    Per-token gates and b2 are intentionally NOT applied here (host does
    that); this keeps every device instruction to <=1 sync wait.
    """
    W = Ca + Cd
    nc = bacc.Bacc(
        "TRN2", target_bir_lowering=False, debug=False, num_devices=N_CORES
    )
    xT = nc.declare_dram_parameter("xT", [D, W], bf16, isOutput=False)
    w1pA = nc.declare_dram_parameter("w1pA", [nF, P, nD * P], bf16, isOutput=False)
    w2A = nc.declare_dram_parameter("w2A", [F, D], bf16, isOutput=False)
    b1pA = nc.declare_dram_parameter("b1pA", [P, nF], f32, isOutput=False)
    if Cd:
        w1pT = nc.declare_dram_parameter(
            "w1pT", [nFf, P, nD * P], bf16, isOutput=False
        )
        w2T = nc.declare_dram_parameter("w2T", [nFf * P, D], bf16, isOutput=False)
        b1pT = nc.declare_dram_parameter("b1pT", [P, nFf], f32, isOutput=False)
    yT = nc.declare_dram_parameter("yT", [D, W], bf16, isOutput=True)

    TOK_CHUNK = 512
    chunks = _chunks_of(Ca, TOK_CHUNK)

    with tile.TileContext(nc) as tc:
        with (
            tc.tile_pool(name="const", bufs=1) as const_pool,
            tc.tile_pool(name="hpool", bufs=1) as hpool,
            tc.tile_pool(name="psum1", bufs=4, space="PSUM") as psum1,
            tc.tile_pool(name="psum2", bufs=3, space="PSUM") as psum2,
            tc.tile_pool(name="outp", bufs=2) as outp,
        ):
            # ---- input DMAs, ordered by first use ------------------------
            xT_r = xT.rearrange("(o p) t -> p o t", p=P)
            yT_r = yT.rearrange("(o p) t -> p o t", p=P)
            x_sb = []
            t0, cs = chunks[0]
            xt = const_pool.tile([P, nD, cs], bf16, tag="x_0")
            nc.sync.dma_start(xt[:], xT_r[:, :, t0 : t0 + cs])
            x_sb.append(xt)

            # w1A in fine-grained groups: chunk-0's layer-1 consumes fi
            # tiles at ~0.64us each right after the clock ramp, so each
            # group's completion semaphore must fire early - coarse groups
            # arrive too late and stall the PE.
            W1G = 2
            w1a_sb = []
            for g in range(nF // W1G):
                t = const_pool.tile([P, W1G, nD * P], bf16, tag=f"w1a_{g}")
                nc.sync.dma_start(
                    t[:], w1pA[g * W1G : (g + 1) * W1G].rearrange("f p dc -> p f dc")
                )
                w1a_sb.append(t)
                if g == 0:
                    b1a_sb = const_pool.tile([P, nF], f32)
                    nc.sync.dma_start(b1a_sb[:], b1pA[:, :])
                    if Cd:
                        b1t_sb = const_pool.tile([P, nFf], f32)
                        nc.sync.dma_start(b1t_sb[:], b1pT[:, :])
                    # Pre-touch b1 on the scalar engine so the gelu
                    # activations (which carry the bias as a pointer operand
                    # and thus have only one sync-wait slot) never need to
                    # wait on the DMA.
                    scratch = const_pool.tile([P, 1], f32)
                    nc.scalar.copy(scratch[:], b1a_sb[:, 0:1])
                    if Cd:
                        nc.scalar.copy(scratch[:], b1t_sb[:, 0:1])

            W2G = 4
            w2a_sb = []
            for g in range(nF // W2G):
                t = const_pool.tile([P, W2G, D], bf16, tag=f"w2a_{g}")
                nc.sync.dma_start(
                    t[:],
                    w2A[g * W2G * P : (g + 1) * W2G * P, :].rearrange(
                        "(f p) d -> p f d", p=P
                    ),
                )
                w2a_sb.append(t)

            for ci in range(1, len(chunks)):
                t0, cs = chunks[ci]
                xt = const_pool.tile([P, nD, cs], bf16, tag=f"x_{ci}")
                nc.sync.dma_start(xt[:], xT_r[:, :, t0 : t0 + cs])
                x_sb.append(xt)

            if Cd:
                xtail = const_pool.tile([P, nD, Cd], bf16, tag="x_tail")
                nc.sync.dma_start(xtail[:], xT_r[:, :, Ca : Ca + Cd])
                w1t_sb = const_pool.tile([P, nFf, nD * P], bf16, tag="w1t")
                nc.sync.dma_start(w1t_sb[:], w1pT.rearrange("f p dc -> p f dc"))
                w2t_sb = const_pool.tile([P, nFf, D], bf16, tag="w2t")
                nc.sync.dma_start(
                    w2t_sb[:], w2T.rearrange("(f p) d -> p f d", p=P)
                )

            def w1a_tile(fi, d):
                return w1a_sb[fi // W1G][:, fi % W1G, d * P : (d + 1) * P]

            def w2a_tile(fi, do):
                return w2a_sb[fi // W2G][:, fi % W2G, do * P : (do + 1) * P]

            # Dummy matmuls on a zeroed tile while input DMAs stream in:
            # keeps the PE busy through the HAM activity window so the
            # real matmuls start at 2.4 GHz instead of the cold 1.2 GHz.
            # Any PE idle gap here makes HAM drop the clock to half for
            # ~3.4us right as real work starts, so warmup must bridge all
            # the way to the chunk-0 dependency arrival (~14us).
            warm_src = const_pool.tile([P, P], bf16)
            nc.any.memset(warm_src[:], 0.0)
            for _w in range(38):
                pw = psum1.tile([P, TOK_CHUNK], f32, tag="ph", name="pw")
                for k in range(4):
                    nc.tensor.matmul(
                        pw[:, :64],
                        lhsT=warm_src[:],
                        rhs=warm_src[:, :64],
                        start=(k == 0),
                        stop=(k == 3),
                    )

            for ci, (t0, cs) in enumerate(chunks):
                # h[f, tok] = gelu(sum_d w1[d, f] * x[d, tok] + b1[f])
                h = hpool.tile([P, nF, TOK_CHUNK], bf16, tag="h")
                for fi in range(nF):
                    ph = psum1.tile([P, TOK_CHUNK], f32, tag="ph")
                    for d in range(nD):
                        nc.tensor.matmul(
                            ph[:, :cs],
                            lhsT=w1a_tile(fi, d),
                            rhs=x_sb[ci][:, d, :cs],
                            start=(d == 0),
                            stop=(d == nD - 1),
                        )
                    nc.scalar.activation(
                        h[:, fi, :cs],
                        ph[:, :cs],
                        mybir.ActivationFunctionType.Gelu,
                        bias=b1a_sb[:, fi : fi + 1],
                    )

                # yT[dout, tok] = sum_f w2[f, dout] * h[f, tok]
                # do-major: each dout's psum completes early so its
                # copy-back overlaps the next dout's matmuls. One output
                # tile + one DMA per chunk (not per dout) keeps the Sync
                # queue cost down.
                ot = outp.tile([P, nD, TOK_CHUNK], bf16, tag="ot")
                for do in range(nD):
                    py = psum2.tile([P, TOK_CHUNK], f32, tag="py")
                    for fi in range(nF):
                        nc.tensor.matmul(
                            py[:, :cs],
                            lhsT=w2a_tile(fi, do),
                            rhs=h[:, fi, :cs],
                            start=(fi == 0),
                            stop=(fi == nF - 1),
                        )
                    nc.vector.tensor_copy(ot[:, do, :cs], py[:, :cs])
                nc.sync.dma_start(yT_r[:, :, t0 : t0 + cs], ot[:, :, :cs])

            if Cd:
                # Tail slot: same FFN over nFf F-tiles only (partial y).
                ht = hpool.tile([P, nFf, Cd], bf16, tag="ht")
                for fi in range(nFf):
                    ph = psum1.tile([P, TOK_CHUNK], f32, tag="ph")
                    for d in range(nD):
                        nc.tensor.matmul(
                            ph[:, :Cd],
                            lhsT=w1t_sb[:, fi, d * P : (d + 1) * P],
                            rhs=xtail[:, d, :],
                            start=(d == 0),
                            stop=(d == nD - 1),
                        )
                    nc.scalar.activation(
                        ht[:, fi, :],
                        ph[:, :Cd],
                        mybir.ActivationFunctionType.Gelu,
                        bias=b1t_sb[:, fi : fi + 1],
                    )
                ot = outp.tile([P, nD, TOK_CHUNK], bf16, tag="ot")
                for do in range(nD):
                    py = psum2.tile([P, TOK_CHUNK], f32, tag="py")
                    for fi in range(nFf):
                        nc.tensor.matmul(
                            py[:, :Cd],
                            lhsT=w2t_sb[:, fi, do * P : (do + 1) * P],
                            rhs=ht[:, fi, :],
                            start=(fi == 0),
                            stop=(fi == nFf - 1),
                        )
                    nc.vector.tensor_copy(ot[:, do, :Cd], py[:, :Cd])
                nc.sync.dma_start(yT_r[:, :, Ca : Ca + Cd], ot[:, :, :Cd])
    nc.compile()
    return nc


def _route(xf, router_w, router_b):
    """Top-2 routing, numpy fp32. Returns (idx1, idx2, g1, g2)."""
    logits = xf @ router_w + router_b
    m = logits.max(axis=-1, keepdims=True)
    p = np.exp(logits - m, dtype=np.float32)
    p /= p.sum(axis=-1, keepdims=True)
    # top-2 indices, ties -> lower index first (matches jax.lax.top_k)
    part = np.argpartition(-p, 1, axis=-1)[:, :2]
    pv = np.take_along_axis(p, part, axis=-1)
    swap = (pv[:, 1] > pv[:, 0]) | ((pv[:, 1] == pv[:, 0]) & (part[:, 1] < part[:, 0]))
    i1 = np.where(swap, part[:, 1], part[:, 0])
    i2 = np.where(swap, part[:, 0], part[:, 1])
    p1 = np.take_along_axis(p, i1[:, None], axis=-1)[:, 0]
    p2 = np.take_along_axis(p, i2[:, None], axis=-1)[:, 0]
    s = p1 + p2
    return i1, i2, p1 / s, p2 / s


# PE cost model constants for planning: ns per matmul row at 2.4 GHz,
# and the tensor queue's per-matmul overhead floor for narrow chunks
# (measured 26-66ns depending on psum-group pressure; 40 is typical).
_CYC = 0.4166
_ISSUE = 40.0


def _plan(loads):
    """Choose (Ca, nFf, Cd) minimizing the uniform per-core PE cost.
    Every expert's load fits in its A slot (Ca) plus overflow pieces of
    <=Cd tokens; each piece occupies nF/nFf cores' tail slots (each
    covering nFf of the nF F-tiles), with at most one tail slot per
    core. Cost = 2*nF*nD*Ca*cyc + 2*nFf*nD*max(issue, Cd*cyc)."""
    r8 = lambda v: -(-v // 8) * 8
    lo, hi = max(8, r8(min(loads))), r8(max(loads))
    best = (2 * nF * nD * hi * _CYC, hi, 0, 0)
    for Ca in range(lo, hi + 8, 8):
        base = 2 * nF * nD * Ca * _CYC
        need = [L - Ca for L in loads if L > Ca]
        if not need:
            if base < best[0]:
                best = (base, Ca, 0, 0)
            continue
        for nFf in (nF, nF // 2, nF // 4, nF // 8):
            cpp = nF // nFf
            for Cd in range(8, 513, 8):
                if sum(-(-n // Cd) for n in need) * cpp <= N_CORES:
                    cost = base + 2 * nFf * nD * max(_ISSUE, Cd * _CYC)
                    if cost < best[0]:
                        best = (cost, Ca, nFf, Cd)
    return best[1], best[2], best[3]


def kernel(x, router_w, router_b, w1, b1, w2, b2):
    global last_results
    x = np.asarray(x, dtype=np.float32)
    router_w = np.asarray(router_w, dtype=np.float32)
    router_b = np.asarray(router_b, dtype=np.float32)
    w1 = np.asarray(w1, dtype=np.float32)
    b1 = np.asarray(b1, dtype=np.float32)
    w2 = np.asarray(w2, dtype=np.float32)
    b2 = np.asarray(b2, dtype=np.float32)

    B, S, _ = x.shape
    T = B * S
    xf = x.reshape(T, D)

    i1, i2, g1, g2 = _route(xf, router_w, router_b)

    tok_lists = []
    gate_lists = []
    for e in range(E):
        m1 = i1 == e
        m2 = i2 == e
        toks = np.nonzero(m1 | m2)[0]
        gates = np.where(m1[toks], g1[toks], g2[toks]).astype(np.float32)
        tok_lists.append(toks)
        gate_lists.append(gates)

    loads = [len(t) for t in tok_lists]
    Ca, nFf, Cd = _plan(loads)
    W = Ca + Cd

    # Overflow pieces: expert e's tokens beyond Ca, split into chunks of
    # <=Cd tokens. Piece j gets cores [j*cpp, (j+1)*cpp), core r of the
    # piece covering F-tiles [r*nFf, (r+1)*nFf).
    a_counts = [min(loads[e], Ca) for e in range(E)]
    pieces = []  # (expert, tok_lo, tok_hi)
    for e in range(E):
        n, lo = loads[e] - a_counts[e], a_counts[e]
        while n > 0:
            take = min(n, Cd)
            pieces.append((e, lo, lo + take))
            lo += take
            n -= take
    cpp = (nF // nFf) if Cd else 0
    assert len(pieces) * cpp <= N_CORES

    xf_b = xf.astype(ml_dtypes.bfloat16)

    def pack_w1(e):
        # w1p[fi, p, d*P + c] = w1[d*P + p, fi*P + c]
        w1_b = w1[e].astype(ml_dtypes.bfloat16)
        return np.ascontiguousarray(
            w1_b.reshape(nD, P, nF, P).transpose(2, 1, 0, 3).reshape(nF, P, nD * P)
        )

    w1p_all = [pack_w1(e) for e in range(E)]
    b1p_all = [np.ascontiguousarray(b1[e].reshape(nF, P).T) for e in range(E)]
    w2b_all = [w2[e].astype(ml_dtypes.bfloat16) for e in range(E)]

    # tail slot assignment: core -> (piece_idx, f_range_idx)
    tail_of = [None] * N_CORES
    for j in range(len(pieces)):
        for r in range(cpp):
            tail_of[j * cpp + r] = (j, r)

    in_maps = []
    for core in range(N_CORES):
        toksA = tok_lists[core][: a_counts[core]]
        xT = np.zeros((D, W), dtype=ml_dtypes.bfloat16)
        xT[:, : len(toksA)] = xf_b[toksA].T
        im = {
            "xT": xT,
            "w1pA": w1p_all[core],
            "w2A": w2b_all[core],
            "b1pA": b1p_all[core],
        }
        if Cd:
            if tail_of[core] is not None:
                j, r = tail_of[core]
                eT, lo, hi = pieces[j]
                toksT = tok_lists[eT][lo:hi]
                xT[:, Ca : Ca + len(toksT)] = xf_b[toksT].T
            else:
                eT, r = core, 0
            fsl = slice(r * nFf, (r + 1) * nFf)
            im.update(
                {
                    "w1pT": np.ascontiguousarray(w1p_all[eT][fsl]),
                    "w2T": np.ascontiguousarray(
                        w2b_all[eT][r * nFf * P : (r + 1) * nFf * P]
                    ),
                    "b1pT": np.ascontiguousarray(b1p_all[eT][:, fsl]),
                }
            )
        in_maps.append(im)

    nc = _build(Ca, nFf, Cd)
    trace = bool(int(os.environ.get("KERNEL_TRACE", "0")))
    last_results = run_bass_kernel_spmd(
        nc, in_maps, core_ids=list(range(N_CORES)), trace=trace
    )

    out = np.zeros((T, D), dtype=np.float32)
    for core in range(N_CORES):
        yT = np.asarray(last_results.results[core]["yT"], dtype=np.float32)
        toksA = tok_lists[core][: a_counts[core]]
        gA = gate_lists[core][: a_counts[core]]
        out[toksA] += gA[:, None] * (yT[:, : len(toksA)].T + b2[core][None, :])
    # tail pieces: sum the partial outputs across each piece's core group
    for j, (eT, lo, hi) in enumerate(pieces):
        toksT = tok_lists[eT][lo:hi]
        gT = gate_lists[eT][lo:hi]
        ysum = np.zeros((len(toksT), D), dtype=np.float32)
        for r in range(cpp):
            core = j * cpp + r
            yT = np.asarray(last_results.results[core]["yT"], dtype=np.float32)
            ysum += yT[:, Ca : Ca + len(toksT)].T
        out[toksT] += gT[:, None] * (ysum + b2[eT][None, :])
    return out.reshape(B, S, D)
